# revision 58
# baseline (speedup 1.0000x reference)
"""Causal multi-head attention block on 8 Trainium2 NeuronCores.

Sharding: 8 cores = 4 batches (data parallel) x 2 head-groups (tensor
parallel over heads). Core c handles batch c//2 and global heads
(c%2)*8 .. (c%2)*8+8. Each core computes a partial output projection
(split-K over its 512 head-output channels); the host sums the two
partials per batch and adds b_proj.

Per-core kernel (bf16 operands, fp32 PSUM accumulation):
  inputs:  x = x^T [1024, 2048] bf16 (host pre-transposes the batch),
           wqkv [1152, 1536] bf16 (rows 0..1023 = w_attn cols for this
           core's q|k|v heads, row 1024 = b_attn slice, rest zero),
           wproj [512, 1024] bf16
  output:  out [2048, 1024] fp32 = partial projection

Design notes (vs the fp32r baseline this evolved from):
  - x arrives pre-transposed; x^T strips are contiguous DMA loads.
  - All matmul operands are bf16: 1 cycle/row at any N (exact causal
    trimming of diagonal tiles), and FWL fast weight loads.
  - S^T tiles [j=128, head-pair, i=512] fp32 psum; one Exp per tile;
    inner loop is software-pipelined with skew 2 (S(jj+2) issues ahead
    of PV(jj)) so the in-order PE never drains on the exp semaphore.
  - PV uses M=128 stationary [ones (64 cols) | v_h (64 cols)]: rows
    0..63 of the PV psum replicate the softmax denominator, so the
    reciprocal runs as one custom-DVE reciprocal_approx_fast (ACT
    Ln/Exp would thrash activation table sets; plain DVE reciprocal
    is ~6.4ns/elem/lane; the replica sits base-0 because a shifted
    base corrupts InstCustomDveAnt).
  - b_attn for the q|k strips folds into the psum evacuation as a
    per-partition tensor_scalar_add; the v strip keeps the x_aug
    ones-row augmentation.
  - The PE executes its stream in order, so the PE-dense qkv chains
    and the ACT-paced attention are woven at emission time: attention
    runs head-pair-outer (pair hp needs only strips hp, 4+hp), and a
    filler queue drips one qkv chain / vau tile / proj chunk between
    attention j-tiles. All chain psums share one 3-slot pool; psy (2
    slots) carries the PV accumulators. 8 PSUM banks total, ~97% PE
    occupancy mid-kernel.
"""

import threading
from contextlib import ExitStack

import numpy as np
import ml_dtypes

import concourse.bass as bass
import concourse.mybir as mybir
import concourse.tile as tile
from concourse import bacc
from concourse.bass_utils import run_bass_kernel_spmd

F32 = mybir.dt.float32
BF16 = mybir.dt.bfloat16
NP_BF16 = ml_dtypes.bfloat16

B, T, C = 4, 2048, 1024
H, DH = 16, 64
N_CORES = 8
HL = 8                  # local heads per core
NQK = 2 * HL * DH       # 1024 qkT rows (q 512 | k 512)
NV = HL * DH            # 512 v cols
CS = C // 128           # 8 real c-strips
CS_AUG = CS + 1         # + bias strip
TT = T // 128           # 16 token tiles
TB = T // 512           # 4 token blocks
SCALE = 1.0 / 8.0       # 1/sqrt(DH)
ACT_EXP = mybir.ActivationFunctionType.Exp


def build_attention_kernel(ctx: ExitStack, tc: tile.TileContext,
                           x: bass.AP, wqkv: bass.AP, wproj: bass.AP,
                           bqk: bass.AP, out: bass.AP):
    nc = tc.nc

    const_pool = ctx.enter_context(tc.tile_pool(name="const", bufs=1))
    # causal diag mask: 1 where i >= j (keep), 0 where i < j
    mask01 = const_pool.tile([128, 128], BF16, tag="mask01")
    nc.gpsimd.memset(mask01[:], 1.0)
    nc.gpsimd.affine_select(
        out=mask01[:], in_=mask01[:],
        compare_op=mybir.AluOpType.is_ge, fill=0.0, base=0,
        pattern=[[1, 128]], channel_multiplier=-1)

    # persistent SBUF
    qkt_pool = ctx.enter_context(tc.tile_pool(name="qkt", bufs=1))
    qkt = [qkt_pool.tile([128, T], BF16, tag=f"qkt{s}", name=f"qkt{s}")
           for s in range(NQK // 128)]
    vau_pool = ctx.enter_context(tc.tile_pool(name="vau", bufs=1))
    # [j, h, 0:64] = ones (denominator replicator; base-0 so the
    # custom-DVE reciprocal reads PSUM partitions 0..63 -- a shifted
    # base corrupts InstCustomDveAnt); [j, h, 64:128] = v_h
    vau = [vau_pool.tile([128, HL, 2 * DH], BF16, tag=f"v{tt}",
                         name=f"vau{tt}")
           for tt in range(TT)]
    for tt in range(TT):
        nc.gpsimd.memset(vau[tt][:, :, 0:DH], 1.0)
    yt_pool = ctx.enter_context(tc.tile_pool(name="yt", bufs=1))
    yt = [yt_pool.tile([128, T], BF16, tag=f"yt{s}", name=f"yt{s}")
          for s in range(NV // 128)]

    # x^T strips stay resident (late qkv filler chains still read them)
    xt_pool = ctx.enter_context(tc.tile_pool(name="xt", bufs=1))
    xt = [xt_pool.tile([128, T], BF16, tag=f"xt{s}", name=f"xt{s}")
          for s in range(CS)]

    # ---- phase 1: x^T strips (x is pre-transposed host-side) ----
    # halves spread across more DMA queues; issue alternates between
    # the two HWDGE engines (Sync, ACT) so descriptor generation for
    # the critical x^T stream is not serialized behind one sequencer
    for s in range(CS):
        for h in range(2):
            eng = nc.sync if (2 * s + h) % 2 == 0 else nc.scalar
            eng.dma_start(xt[s][:, h * 1024:(h + 1) * 1024],
                          x[s * 128:(s + 1) * 128,
                            h * 1024:(h + 1) * 1024])
    # b_attn per-partition bias columns for the q|k strips
    bias_qk = const_pool.tile([128, 8], F32, tag="biasqk")
    nc.sync.dma_start(bias_qk[:], bqk.rearrange("(s p) -> p s", p=128))
    # b_attn for the v columns, replicated down all 128 partitions so
    # the vau evacuation can add it as a plain tensor operand
    bv = const_pool.tile([128, NV], BF16, tag="bv")
    nc.sync.dma_start(bv[:], wqkv[C:C + 1, NQK:].broadcast_to([128, NV]))

    # ---- phases 2+3: qkv / attention / proj, filler-interleaved ----
    # PE executes in emission order, so the PE-dense qkv chains and the
    # ACT-paced attention must be woven together: attention runs
    # hp-outer (head-pair hp needs only strips hp, 4+hp), and a filler
    # queue drips one qkv chain / vau tile / proj chunk between
    # attention j-tiles. All chain psums share one 3-slot pool; the
    # psy pool (2 slots) doubles as proj psum. 8 banks total.
    wnn_pool = ctx.enter_context(tc.tile_pool(name="wnn", bufs=1))
    wn = [wnn_pool.tile([128, CS_AUG, 128], BF16, tag=f"wnn{nn}",
                        name=f"wnn{nn}")
          for nn in range(8)]
    wv_pool = ctx.enter_context(tc.tile_pool(name="wv", bufs=1))
    wv = wv_pool.tile([128, CS_AUG, NV], BF16, tag="wv")
    # DMA priority: the first chains need wn0, wn4 and wv; wp is only
    # needed ~200us in
    def wn_dma(nn):
        # per-K-strip: chain matmul s only waits for its own strip
        for s in range(CS_AUG):
            nc.sync.dma_start(
                wn[nn][:, s, :],
                wqkv[s * 128:(s + 1) * 128, nn * 128:(nn + 1) * 128])

    for nn in (0, 4):
        wn_dma(nn)
    for s in range(CS_AUG):
        nc.sync.dma_start(wv[:, s, :], wqkv[s * 128:(s + 1) * 128, NQK:])
    for nn in (1, 5, 2, 6, 3, 7):
        wn_dma(nn)
    wp_pool = ctx.enter_context(tc.tile_pool(name="wp", bufs=1))
    wp = wp_pool.tile([128, NV // 128, C], BF16, tag="wp")
    nc.sync.dma_start(wp[:], wproj.rearrange("(s p) n -> p s n", p=128))

    pt_sb_pool = ctx.enter_context(tc.tile_pool(name="ptile", bufs=6))
    n_sb_pool = ctx.enter_context(tc.tile_pool(name="ntile", bufs=3))
    osb_pool = ctx.enter_context(tc.tile_pool(name="osb", bufs=2))
    ps_s_pool = ctx.enter_context(
        tc.tile_pool(name="ps_s", bufs=3, space="PSUM"))
    ps_y_pool = ctx.enter_context(
        tc.tile_pool(name="ps_y", bufs=1, space="PSUM"))

    def qk_chain(nn, tb):
        # one [128, 512] block of qkT strip nn (borrows a pss slot)
        ps = ps_s_pool.tile([128, 2, 512], F32, tag="pss",
                            name=f"pqk{nn}_{tb}")
        for s in range(CS):
            nc.tensor.matmul(ps[:, 0, :], wn[nn][:, s, :],
                             xt[s][:, tb * 512:(tb + 1) * 512],
                             start=(s == 0), stop=(s == CS - 1))
        # evacuate with the b_attn bias folded in (per-partition)
        nc.vector.tensor_scalar_add(
            qkt[nn][:, tb * 512:(tb + 1) * 512], ps[:, 0, :],
            bias_qk[:, nn:nn + 1])

    def vau_tile(tt):
        ps = ps_s_pool.tile([128, 2, 512], F32, tag="pss",
                            name=f"pv{tt}")
        for s in range(CS):
            nc.tensor.matmul(ps[:, 0, :],
                             xt[s][:, tt * 128:(tt + 1) * 128],
                             wv[:, s, :],
                             start=(s == 0), stop=(s == CS - 1))
        # evacuate with b_attn(v) added elementwise
        nc.vector.scalar_tensor_tensor(
            vau[tt][:, :, DH:],
            ps[:, 0, :].rearrange("p (h d) -> p h d", d=DH),
            0.0,
            bv[:].rearrange("p (h d) -> p h d", d=DH),
            mybir.AluOpType.add,
            mybir.AluOpType.add)

    osb = [osb_pool.tile([128, C], F32, tag=f"osb{i}", name=f"osb{i}")
           for i in range(2)]

    def proj_chunk(tt, nb):
        ps = ps_s_pool.tile([128, 2, 512], F32, tag="pss",
                            name=f"po{tt}_{nb}")
        for s in range(NV // 128):
            nc.tensor.matmul(
                ps[:, 0, :],
                yt[s][:, tt * 128:(tt + 1) * 128],
                wp[:, s, nb * 512:(nb + 1) * 512],
                start=(s == 0), stop=(s == NV // 128 - 1))
        o_sb = osb[tt % 2]
        nc.vector.tensor_copy(o_sb[:, nb * 512:(nb + 1) * 512],
                              ps[:, 0, :])
        if nb == C // 512 - 1:
            nc.sync.dma_start(out[tt * 128:(tt + 1) * 128, :], o_sb[:])

    def attn_gen(ib, hp):
        # generator: yields after each j-tile so fillers can interleave
        isl = slice(ib * 512, (ib + 1) * 512)
        jmax = 4 * ib + 3
        qs = qkt[hp]              # q strip: heads (2hp, 2hp+1)
        ks = qkt[4 + hp]          # k strip
        ps_y = [ps_y_pool.tile([128, 512], F32, tag=f"psy{u}",
                               name=f"psy{u}_{hp}_{ib}")
                for u in range(2)]

        def s_exp(jj):
            off = max(0, 128 * (jj - 4 * ib))
            ps_s = ps_s_pool.tile([128, 2, 512], F32, tag="pss")
            for u in range(2):   # head-pair halves: base 0 / 64
                plo = 64 * u
                nc.tensor.matmul(
                    ps_s[:, u, off:],
                    ks[plo:plo + DH, jj * 128:(jj + 1) * 128],
                    qs[plo:plo + DH, ib * 512 + off:(ib + 1) * 512],
                    start=True, stop=True)
            p = pt_sb_pool.tile([128, 2, 512], BF16, tag="pt")
            nc.scalar.activation(p[:, :, off:], ps_s[:, :, off:],
                                 ACT_EXP, scale=SCALE)
            if jj >= 4 * ib:       # diagonal tile: zero i < j
                nc.vector.tensor_mul(
                    p[:, :, off:off + 128],
                    p[:, :, off:off + 128],
                    mask01[:, None, :].broadcast_to([128, 2, 128]))
            return p

        def pv(jj, p):
            off = max(0, 128 * (jj - 4 * ib))
            for u in range(2):
                nc.tensor.matmul(ps_y[u][:, off:],
                                 vau[jj][:, 2 * hp + u, :],
                                 p[:, u, off:],
                                 start=(jj == 0), stop=(jj == jmax))

        # software pipeline, skew 2: S(jj+2) issues before PV(jj)
        p0 = s_exp(0)
        yield
        p1 = s_exp(1)
        yield
        for jj in range(2, jmax + 1):
            p2 = s_exp(jj)
            pv(jj - 2, p0)
            p0, p1 = p1, p2
            yield
        pv(jmax - 1, p0)
        pv(jmax, p1)
        for u in range(2):
            plo = 64 * u
            rbb = n_sb_pool.tile([64, 512], F32, tag=f"rbb{u}")
            nc.vector.reciprocal_approx_fast(
                out=rbb[:], in_=ps_y[u][0:64, :])
            nc.vector.tensor_mul(yt[hp][plo:plo + DH, isl],
                                 ps_y[u][64:128, :], rbb[:])

    # lead-in: ONLY what attention block (hp0, ib0) needs -- the tb0
    # chains of strips 0/4 and the first four v tiles. Everything else
    # weaves into the attention stream, deadline-ordered.
    qk_chain(0, 0)
    qk_chain(4, 0)
    for tt in range(0, 4):
        vau_tile(tt)

    # filler queue, ordered by when attention first needs each item:
    # hp0's own later token-block chains + v tiles come first
    fillers = []
    for tb in range(1, TB):
        fillers.append(lambda tb=tb: qk_chain(0, tb))
        fillers.append(lambda tb=tb: qk_chain(4, tb))
        for tt in range(4 * tb, 4 * tb + 4):
            fillers.append(lambda tt=tt: vau_tile(tt))
    for hp_next in (1, 2, 3):
        for nn in (hp_next, 4 + hp_next):
            for tb in range(TB):
                fillers.append(lambda nn=nn, tb=tb: qk_chain(nn, tb))

    fi = 0
    tick = 0
    for hp in range(HL // 2):
        for ib in range(TB):
            rate = (1, 2, 2, 1)[hp]
            for _ in attn_gen(ib, hp):
                tick += 1
                if tick % rate == 0 and fi < len(fillers):
                    fillers[fi]()
                    fi += 1
            if hp == HL // 2 - 1:
                # this i-block's projection unlocks once hp3 finishes it
                for tt in range(4 * ib, 4 * ib + 4):
                    for nb in range(C // 512):
                        fillers.append(
                            lambda tt=tt, nb=nb: proj_chunk(tt, nb))
    while fi < len(fillers):   # drain: remaining proj chunks
        fillers[fi]()
        fi += 1

_BUILD_LOCK = threading.Lock()
_CACHED = {}


def build_nc(repeat=1):
    with _BUILD_LOCK:
        if repeat in _CACHED:
            return _CACHED[repeat]
        nc = bacc.Bacc("TRN2", debug=False)
        x = nc.dram_tensor("x", [C, T], BF16, kind="ExternalInput").ap()
        wqkv = nc.dram_tensor("wqkv", [CS_AUG * 128, 3 * NV], BF16,
                              kind="ExternalInput").ap()
        wproj = nc.dram_tensor("wproj", [NV, C], BF16,
                               kind="ExternalInput").ap()
        bqk = nc.dram_tensor("bqk", [NQK], F32, kind="ExternalInput").ap()
        out = nc.dram_tensor("out", [T, C], F32, kind="ExternalOutput").ap()
        with tile.TileContext(nc, pool_alloc_mode="queue") as tc:
            for _ in range(repeat):
                with ExitStack() as ctx:
                    build_attention_kernel(ctx, tc, x, wqkv, wproj, bqk, out)
        nc.compile()
        _CACHED[repeat] = nc
        return nc


def shard_inputs(x, w_attn, b_attn, w_proj, b_proj):
    """Build the per-core input maps (numpy, bf16)."""
    x = np.asarray(x, dtype=np.float32)
    w_attn = np.asarray(w_attn, dtype=np.float32)
    b_attn = np.asarray(b_attn, dtype=np.float32)
    w_proj = np.asarray(w_proj, dtype=np.float32)
    in_maps = []
    for c in range(N_CORES):
        b, hh = divmod(c, 2)
        cols = np.r_[hh * 512:(hh + 1) * 512,
                     C + hh * 512:C + (hh + 1) * 512,
                     2 * C + hh * 512:2 * C + (hh + 1) * 512]
        w_aug = np.zeros((CS_AUG * 128, 3 * NV), np.float32)
        w_aug[:C] = w_attn[:, cols]
        w_aug[C] = b_attn[cols]
        in_maps.append({
            "x": np.ascontiguousarray(x[b].T).astype(NP_BF16),
            "wqkv": w_aug.astype(NP_BF16),
            "wproj": np.ascontiguousarray(
                w_proj[hh * 512:(hh + 1) * 512]).astype(NP_BF16),
            "bqk": np.ascontiguousarray(b_attn[cols[:NQK]]),
        })
    return in_maps


def kernel(x, w_attn, b_attn, w_proj, b_proj, _profile=False, _tmpdir=None):
    nc = build_nc()
    in_maps = shard_inputs(x, w_attn, b_attn, w_proj, b_proj)
    res = run_bass_kernel_spmd(nc, in_maps, list(range(N_CORES)),
                               trace=_profile, tmpdir=_tmpdir)
    b_proj = np.asarray(b_proj, dtype=np.float32)
    out = np.empty((B, T, C), np.float32)
    for b in range(B):
        out[b] = res.results[2 * b]["out"] + res.results[2 * b + 1]["out"] \
            + b_proj[None, :]
    if _profile:
        return out, res
    return out


# revision 59
# speedup vs baseline: 1.0418x; 1.0418x over previous
"""Causal multi-head attention block on 8 Trainium2 NeuronCores.

Sharding: 8 cores = 4 batches (data parallel) x 2 head-groups (tensor
parallel over heads). Core c handles batch c//2 and global heads
(c%2)*8 .. (c%2)*8+8. Each core computes a partial output projection
(split-K over its 512 head-output channels); the host sums the two
partials per batch and adds b_proj.

Per-core kernel (bf16 operands, fp32 PSUM accumulation):
  inputs:  x = x^T [1024, 2048] bf16 (host pre-transposes the batch),
           wqkv [1152, 1536] bf16 (rows 0..1023 = w_attn cols for this
           core's q|k|v heads, row 1024 = b_attn slice, rest zero),
           wproj [512, 1024] bf16
  output:  out [2048, 1024] fp32 = partial projection

Design notes (vs the fp32r baseline this evolved from):
  - x arrives pre-transposed; x^T strips are contiguous DMA loads.
  - All matmul operands are bf16: 1 cycle/row at any N (exact causal
    trimming of diagonal tiles), and FWL fast weight loads.
  - S^T tiles [j=128, head-pair, i=512] fp32 psum; one Exp per tile;
    inner loop is software-pipelined with skew 2 (S(jj+2) issues ahead
    of PV(jj)) so the in-order PE never drains on the exp semaphore.
  - PV uses M=128 stationary [ones (64 cols) | v_h (64 cols)]: rows
    0..63 of the PV psum replicate the softmax denominator, so the
    reciprocal runs as one custom-DVE reciprocal_approx_fast (ACT
    Ln/Exp would thrash activation table sets; plain DVE reciprocal
    is ~6.4ns/elem/lane; the replica sits base-0 because a shifted
    base corrupts InstCustomDveAnt).
  - b_attn for the q|k strips folds into the psum evacuation as a
    per-partition tensor_scalar_add; the v strip keeps the x_aug
    ones-row augmentation.
  - The PE executes its stream in order, so the PE-dense qkv chains
    and the ACT-paced attention are woven at emission time: attention
    runs head-pair-outer (pair hp needs only strips hp, 4+hp), and a
    filler queue drips one qkv chain / vau tile / proj chunk between
    attention j-tiles. All chain psums share one 3-slot pool; psy (2
    slots) carries the PV accumulators. 8 PSUM banks total, ~97% PE
    occupancy mid-kernel.
"""

import threading
from contextlib import ExitStack

import numpy as np
import ml_dtypes

import concourse.bass as bass
import concourse.mybir as mybir
import concourse.tile as tile
from concourse import bacc
from concourse.bass_utils import run_bass_kernel_spmd

F32 = mybir.dt.float32
BF16 = mybir.dt.bfloat16
NP_BF16 = ml_dtypes.bfloat16

B, T, C = 4, 2048, 1024
H, DH = 16, 64
N_CORES = 8
HL = 8                  # local heads per core
NQK = 2 * HL * DH       # 1024 qkT rows (q 512 | k 512)
NV = HL * DH            # 512 v cols
CS = C // 128           # 8 real c-strips
CS_AUG = CS + 1         # + bias strip
TT = T // 128           # 16 token tiles
TB = T // 512           # 4 token blocks
SCALE = 1.0 / 8.0       # 1/sqrt(DH)
ACT_EXP = mybir.ActivationFunctionType.Exp


def build_attention_kernel(ctx: ExitStack, tc: tile.TileContext,
                           x: bass.AP, wqkv: bass.AP, wproj: bass.AP,
                           bqk: bass.AP, out: bass.AP):
    nc = tc.nc

    const_pool = ctx.enter_context(tc.tile_pool(name="const", bufs=1))
    # causal diag mask: 1 where i >= j (keep), 0 where i < j
    mask01 = const_pool.tile([128, 128], BF16, tag="mask01")
    nc.gpsimd.memset(mask01[:], 1.0)
    nc.gpsimd.affine_select(
        out=mask01[:], in_=mask01[:],
        compare_op=mybir.AluOpType.is_ge, fill=0.0, base=0,
        pattern=[[1, 128]], channel_multiplier=-1)

    # persistent SBUF
    qkt_pool = ctx.enter_context(tc.tile_pool(name="qkt", bufs=1))
    qkt = [qkt_pool.tile([128, T], BF16, tag=f"qkt{s}", name=f"qkt{s}")
           for s in range(NQK // 128)]
    vau_pool = ctx.enter_context(tc.tile_pool(name="vau", bufs=1))
    # [j, h, 0:64] = ones (denominator replicator; base-0 so the
    # custom-DVE reciprocal reads PSUM partitions 0..63 -- a shifted
    # base corrupts InstCustomDveAnt); [j, h, 64:128] = v_h
    vau = [vau_pool.tile([128, HL, 2 * DH], BF16, tag=f"v{tt}",
                         name=f"vau{tt}")
           for tt in range(TT)]
    for tt in range(TT):
        nc.gpsimd.memset(vau[tt][:, :, 0:DH], 1.0)
    yt_pool = ctx.enter_context(tc.tile_pool(name="yt", bufs=1))
    yt = [yt_pool.tile([128, T], BF16, tag=f"yt{s}", name=f"yt{s}")
          for s in range(NV // 128)]

    # x^T strips stay resident (late qkv filler chains still read them)
    xt_pool = ctx.enter_context(tc.tile_pool(name="xt", bufs=1))
    xt = [xt_pool.tile([128, T], BF16, tag=f"xt{s}", name=f"xt{s}")
          for s in range(CS)]

    # ---- phase 1: x^T strips (x is pre-transposed host-side) ----
    # halves spread across more DMA queues; issue alternates between
    # the two HWDGE engines (Sync, ACT) so descriptor generation for
    # the critical x^T stream is not serialized behind one sequencer
    for s in range(CS):
        for h in range(2):
            eng = nc.sync if (2 * s + h) % 2 == 0 else nc.scalar
            eng.dma_start(xt[s][:, h * 1024:(h + 1) * 1024],
                          x[s * 128:(s + 1) * 128,
                            h * 1024:(h + 1) * 1024])
    # b_attn per-partition bias columns for the q|k strips
    bias_qk = const_pool.tile([128, 8], F32, tag="biasqk")
    nc.sync.dma_start(bias_qk[:], bqk.rearrange("(s p) -> p s", p=128))
    # b_attn for the v columns, replicated down all 128 partitions so
    # the vau evacuation can add it as a plain tensor operand
    bv = const_pool.tile([128, NV], BF16, tag="bv")
    nc.sync.dma_start(bv[:], wqkv[C:C + 1, NQK:].broadcast_to([128, NV]))

    # ---- phases 2+3: qkv / attention / proj, filler-interleaved ----
    # PE executes in emission order, so the PE-dense qkv chains and the
    # ACT-paced attention must be woven together: attention runs
    # hp-outer (head-pair hp needs only strips hp, 4+hp), and a filler
    # queue drips one qkv chain / vau tile / proj chunk between
    # attention j-tiles. All chain psums share one 3-slot pool; the
    # psy pool (2 slots) doubles as proj psum. 8 banks total.
    wnn_pool = ctx.enter_context(tc.tile_pool(name="wnn", bufs=1))
    wn = [wnn_pool.tile([128, CS_AUG, 128], BF16, tag=f"wnn{nn}",
                        name=f"wnn{nn}")
          for nn in range(8)]
    wv_pool = ctx.enter_context(tc.tile_pool(name="wv", bufs=1))
    wv = wv_pool.tile([128, CS_AUG, NV], BF16, tag="wv")
    # DMA priority: the first chains need wn0, wn4 and wv; wp is only
    # needed ~200us in
    def wn_dma(nn):
        # per-K-strip: chain matmul s only waits for its own strip
        for s in range(CS_AUG):
            nc.sync.dma_start(
                wn[nn][:, s, :],
                wqkv[s * 128:(s + 1) * 128, nn * 128:(nn + 1) * 128])

    for nn in (0, 4):
        wn_dma(nn)
    for s in range(CS_AUG):
        nc.sync.dma_start(wv[:, s, :], wqkv[s * 128:(s + 1) * 128, NQK:])
    for nn in (1, 5, 2, 6, 3, 7):
        wn_dma(nn)
    wp_pool = ctx.enter_context(tc.tile_pool(name="wp", bufs=1))
    wp = wp_pool.tile([128, NV // 128, C], BF16, tag="wp")
    nc.sync.dma_start(wp[:], wproj.rearrange("(s p) n -> p s n", p=128))

    pt_sb_pool = ctx.enter_context(tc.tile_pool(name="ptile", bufs=6))
    n_sb_pool = ctx.enter_context(tc.tile_pool(name="ntile", bufs=3))
    osb_pool = ctx.enter_context(tc.tile_pool(name="osb", bufs=2))
    ps_s_pool = ctx.enter_context(
        tc.tile_pool(name="ps_s", bufs=3, space="PSUM"))
    ps_y_pool = ctx.enter_context(
        tc.tile_pool(name="ps_y", bufs=1, space="PSUM"))

    def qk_chain(nn, tb):
        # one [128, 512] block of qkT strip nn (borrows a pss slot)
        ps = ps_s_pool.tile([128, 2, 512], F32, tag="pss",
                            name=f"pqk{nn}_{tb}")
        for s in range(CS):
            nc.tensor.matmul(ps[:, 0, :], wn[nn][:, s, :],
                             xt[s][:, tb * 512:(tb + 1) * 512],
                             start=(s == 0), stop=(s == CS - 1))
        # evacuate with the b_attn bias folded in (per-partition)
        nc.vector.tensor_scalar_add(
            qkt[nn][:, tb * 512:(tb + 1) * 512], ps[:, 0, :],
            bias_qk[:, nn:nn + 1])

    def vau_tile(tt):
        ps = ps_s_pool.tile([128, 2, 512], F32, tag="pss",
                            name=f"pv{tt}")
        for s in range(CS):
            nc.tensor.matmul(ps[:, 0, :],
                             xt[s][:, tt * 128:(tt + 1) * 128],
                             wv[:, s, :],
                             start=(s == 0), stop=(s == CS - 1))
        # evacuate with b_attn(v) added elementwise
        nc.vector.scalar_tensor_tensor(
            vau[tt][:, :, DH:],
            ps[:, 0, :].rearrange("p (h d) -> p h d", d=DH),
            0.0,
            bv[:].rearrange("p (h d) -> p h d", d=DH),
            mybir.AluOpType.add,
            mybir.AluOpType.add)

    osb = [osb_pool.tile([128, C], F32, tag=f"osb{i}", name=f"osb{i}")
           for i in range(2)]

    def proj_chunk(tt, nb):
        ps = ps_s_pool.tile([128, 2, 512], F32, tag="pss",
                            name=f"po{tt}_{nb}")
        for s in range(NV // 128):
            nc.tensor.matmul(
                ps[:, 0, :],
                yt[s][:, tt * 128:(tt + 1) * 128],
                wp[:, s, nb * 512:(nb + 1) * 512],
                start=(s == 0), stop=(s == NV // 128 - 1))
        o_sb = osb[tt % 2]
        nc.vector.tensor_copy(o_sb[:, nb * 512:(nb + 1) * 512],
                              ps[:, 0, :])
        if nb == C // 512 - 1:
            nc.sync.dma_start(out[tt * 128:(tt + 1) * 128, :], o_sb[:])

    def attn_gen(ib, hp):
        # generator: yields after each j-tile so fillers can interleave
        isl = slice(ib * 512, (ib + 1) * 512)
        jmax = 4 * ib + 3
        qs = qkt[hp]              # q strip: heads (2hp, 2hp+1)
        ks = qkt[4 + hp]          # k strip
        ps_y = [ps_y_pool.tile([128, 512], F32, tag=f"psy{u}",
                               name=f"psy{u}_{hp}_{ib}")
                for u in range(2)]

        def s_exp(jj):
            off = max(0, 128 * (jj - 4 * ib))
            ps_s = ps_s_pool.tile([128, 2, 512], F32, tag="pss")
            for u in range(2):   # head-pair halves: base 0 / 64
                plo = 64 * u
                nc.tensor.matmul(
                    ps_s[:, u, off:],
                    ks[plo:plo + DH, jj * 128:(jj + 1) * 128],
                    qs[plo:plo + DH, ib * 512 + off:(ib + 1) * 512],
                    start=True, stop=True)
            p = pt_sb_pool.tile([128, 2, 512], BF16, tag="pt")
            nc.scalar.activation(p[:, :, off:], ps_s[:, :, off:],
                                 ACT_EXP, scale=SCALE)
            if jj >= 4 * ib:       # diagonal tile: zero i < j
                nc.vector.tensor_mul(
                    p[:, :, off:off + 128],
                    p[:, :, off:off + 128],
                    mask01[:, None, :].broadcast_to([128, 2, 128]))
            return p

        def pv(jj, p):
            off = max(0, 128 * (jj - 4 * ib))
            for u in range(2):
                nc.tensor.matmul(ps_y[u][:, off:],
                                 vau[jj][:, 2 * hp + u, :],
                                 p[:, u, off:],
                                 start=(jj == 0), stop=(jj == jmax))

        # software pipeline, skew 2: S(jj+2) issues before PV(jj)
        p0 = s_exp(0)
        yield
        p1 = s_exp(1)
        yield
        for jj in range(2, jmax + 1):
            p2 = s_exp(jj)
            pv(jj - 2, p0)
            p0, p1 = p1, p2
            yield
        pv(jmax - 1, p0)
        pv(jmax, p1)
        for u in range(2):
            plo = 64 * u
            rbb = n_sb_pool.tile([64, 512], F32, tag=f"rbb{u}")
            nc.vector.reciprocal_approx_fast(
                out=rbb[:], in_=ps_y[u][0:64, :])
            nc.vector.tensor_mul(yt[hp][plo:plo + DH, isl],
                                 ps_y[u][64:128, :], rbb[:])

    # lead-in: strips for hp=0 + the first vau tiles (ACT idle anyway)
    for tb in range(TB):
        qk_chain(0, tb)
    for tb in range(TB):
        qk_chain(4, tb)
    for tt in range(0, 4):
        vau_tile(tt)

    # filler queue, ordered by when attention first needs each item
    fillers = []
    for tt in range(4, TT):
        fillers.append(lambda tt=tt: vau_tile(tt))
    for hp_next in (1, 2, 3):
        for nn in (hp_next, 4 + hp_next):
            for tb in range(TB):
                fillers.append(lambda nn=nn, tb=tb: qk_chain(nn, tb))

    fi = 0
    tick = 0
    for hp in range(HL // 2):
        for ib in range(TB):
            rate = (2, 3, 2, 1)[hp]
            for _ in attn_gen(ib, hp):
                tick += 1
                if tick % rate == 0 and fi < len(fillers):
                    fillers[fi]()
                    fi += 1
            if hp == HL // 2 - 1:
                # this i-block's projection unlocks once hp3 finishes it
                for tt in range(4 * ib, 4 * ib + 4):
                    for nb in range(C // 512):
                        fillers.append(
                            lambda tt=tt, nb=nb: proj_chunk(tt, nb))
    while fi < len(fillers):   # drain: remaining proj chunks
        fillers[fi]()
        fi += 1

_BUILD_LOCK = threading.Lock()
_CACHED = {}


def build_nc(repeat=1):
    with _BUILD_LOCK:
        if repeat in _CACHED:
            return _CACHED[repeat]
        nc = bacc.Bacc("TRN2", debug=False)
        x = nc.dram_tensor("x", [C, T], BF16, kind="ExternalInput").ap()
        wqkv = nc.dram_tensor("wqkv", [CS_AUG * 128, 3 * NV], BF16,
                              kind="ExternalInput").ap()
        wproj = nc.dram_tensor("wproj", [NV, C], BF16,
                               kind="ExternalInput").ap()
        bqk = nc.dram_tensor("bqk", [NQK], F32, kind="ExternalInput").ap()
        out = nc.dram_tensor("out", [T, C], F32, kind="ExternalOutput").ap()
        with tile.TileContext(nc, pool_alloc_mode="queue") as tc:
            for _ in range(repeat):
                with ExitStack() as ctx:
                    build_attention_kernel(ctx, tc, x, wqkv, wproj, bqk, out)
        nc.compile()
        _CACHED[repeat] = nc
        return nc


def shard_inputs(x, w_attn, b_attn, w_proj, b_proj):
    """Build the per-core input maps (numpy, bf16)."""
    x = np.asarray(x, dtype=np.float32)
    w_attn = np.asarray(w_attn, dtype=np.float32)
    b_attn = np.asarray(b_attn, dtype=np.float32)
    w_proj = np.asarray(w_proj, dtype=np.float32)
    in_maps = []
    for c in range(N_CORES):
        b, hh = divmod(c, 2)
        cols = np.r_[hh * 512:(hh + 1) * 512,
                     C + hh * 512:C + (hh + 1) * 512,
                     2 * C + hh * 512:2 * C + (hh + 1) * 512]
        w_aug = np.zeros((CS_AUG * 128, 3 * NV), np.float32)
        w_aug[:C] = w_attn[:, cols]
        w_aug[C] = b_attn[cols]
        in_maps.append({
            "x": np.ascontiguousarray(x[b].T).astype(NP_BF16),
            "wqkv": w_aug.astype(NP_BF16),
            "wproj": np.ascontiguousarray(
                w_proj[hh * 512:(hh + 1) * 512]).astype(NP_BF16),
            "bqk": np.ascontiguousarray(b_attn[cols[:NQK]]),
        })
    return in_maps


def kernel(x, w_attn, b_attn, w_proj, b_proj, _profile=False, _tmpdir=None):
    nc = build_nc()
    in_maps = shard_inputs(x, w_attn, b_attn, w_proj, b_proj)
    res = run_bass_kernel_spmd(nc, in_maps, list(range(N_CORES)),
                               trace=_profile, tmpdir=_tmpdir)
    b_proj = np.asarray(b_proj, dtype=np.float32)
    out = np.empty((B, T, C), np.float32)
    for b in range(B):
        out[b] = res.results[2 * b]["out"] + res.results[2 * b + 1]["out"] \
            + b_proj[None, :]
    if _profile:
        return out, res
    return out


# revision 60
# speedup vs baseline: 1.2509x; 1.2007x over previous
"""Causal multi-head attention block on 8 Trainium2 NeuronCores.

Sharding: 8 cores = 4 batches (data parallel) x 2 head-groups (tensor
parallel over heads). Core c handles batch c//2 and global heads
(c%2)*8 .. (c%2)*8+8. Each core computes a partial output projection
(split-K over its 512 head-output channels); the host sums the two
partials per batch and adds b_proj.

Per-core kernel (bf16 operands, fp32 PSUM accumulation):
  inputs:  x = x^T [1024, 2048] bf16 (host pre-transposes the batch),
           wqkv [1152, 1536] bf16 (rows 0..1023 = w_attn cols for this
           core's q|k|v heads, row 1024 = b_attn slice, rest zero),
           wproj [512, 1024] bf16
  output:  out [2048, 1024] fp32 = partial projection

Design notes (vs the fp32r baseline this evolved from):
  - x arrives pre-transposed; x^T strips are contiguous DMA loads.
  - All matmul operands are bf16: 1 cycle/row at any N (exact causal
    trimming of diagonal tiles), and FWL fast weight loads.
  - S^T tiles [j=128, head-pair, i=512] fp32 psum; one Exp per tile;
    inner loop is software-pipelined with skew 2 (S(jj+2) issues ahead
    of PV(jj)) so the in-order PE never drains on the exp semaphore.
  - PV uses M=128 stationary [ones (64 cols) | v_h (64 cols)]: rows
    0..63 of the PV psum replicate the softmax denominator, so the
    reciprocal runs as one custom-DVE reciprocal_approx_fast (ACT
    Ln/Exp would thrash activation table sets; plain DVE reciprocal
    is ~6.4ns/elem/lane; the replica sits base-0 because a shifted
    base corrupts InstCustomDveAnt).
  - b_attn for the q|k strips folds into the psum evacuation as a
    per-partition tensor_scalar_add; the v strip keeps the x_aug
    ones-row augmentation.
  - The PE executes its stream in order, so the PE-dense qkv chains
    and the ACT-paced attention are woven at emission time: attention
    runs head-pair-outer (pair hp needs only strips hp, 4+hp), and a
    filler queue drips one qkv chain / vau tile / proj chunk between
    attention j-tiles. All chain psums share one 3-slot pool; psy (2
    slots) carries the PV accumulators. 8 PSUM banks total, ~97% PE
    occupancy mid-kernel.
"""

import threading
from contextlib import ExitStack

import numpy as np
import ml_dtypes

import concourse.bass as bass
import concourse.mybir as mybir
import concourse.tile as tile
from concourse import bacc
from concourse.bass_utils import run_bass_kernel_spmd

F32 = mybir.dt.float32
BF16 = mybir.dt.bfloat16
NP_BF16 = ml_dtypes.bfloat16

B, T, C = 4, 2048, 1024
H, DH = 16, 64
N_CORES = 8
HL = 8                  # local heads per core
NQK = 2 * HL * DH       # 1024 qkT rows (q 512 | k 512)
NV = HL * DH            # 512 v cols
CS = C // 128           # 8 real c-strips
CS_AUG = CS + 1         # + bias strip
TT = T // 128           # 16 token tiles
TB = T // 512           # 4 token blocks
SCALE = 1.0 / 8.0       # 1/sqrt(DH)
ACT_EXP = mybir.ActivationFunctionType.Exp


def build_attention_kernel(ctx: ExitStack, tc: tile.TileContext,
                           x: bass.AP, wqkv: bass.AP, wproj: bass.AP,
                           bqk: bass.AP, out: bass.AP):
    nc = tc.nc

    const_pool = ctx.enter_context(tc.tile_pool(name="const", bufs=1))
    # causal diag mask: 1 where i >= j (keep), 0 where i < j
    mask01 = const_pool.tile([128, 128], BF16, tag="mask01")
    nc.gpsimd.memset(mask01[:], 1.0)
    nc.gpsimd.affine_select(
        out=mask01[:], in_=mask01[:],
        compare_op=mybir.AluOpType.is_ge, fill=0.0, base=0,
        pattern=[[1, 128]], channel_multiplier=-1)

    # persistent SBUF
    qkt_pool = ctx.enter_context(tc.tile_pool(name="qkt", bufs=1))
    qkt = [qkt_pool.tile([128, T], BF16, tag=f"qkt{s}", name=f"qkt{s}")
           for s in range(NQK // 128)]
    vau_pool = ctx.enter_context(tc.tile_pool(name="vau", bufs=1))
    # [j, h, 0:64] = ones (denominator replicator; base-0 so the
    # custom-DVE reciprocal reads PSUM partitions 0..63 -- a shifted
    # base corrupts InstCustomDveAnt); [j, h, 64:128] = v_h
    vau = [vau_pool.tile([128, HL, 2 * DH], BF16, tag=f"v{tt}",
                         name=f"vau{tt}")
           for tt in range(TT)]
    for tt in range(TT):
        nc.gpsimd.memset(vau[tt][:, :, 0:DH], 1.0)
    yt_pool = ctx.enter_context(tc.tile_pool(name="yt", bufs=1))
    yt = [yt_pool.tile([128, T], BF16, tag=f"yt{s}", name=f"yt{s}")
          for s in range(NV // 128)]

    # x^T strips stay resident (late qkv filler chains still read them)
    xt_pool = ctx.enter_context(tc.tile_pool(name="xt", bufs=1))
    xt = [xt_pool.tile([128, T], BF16, tag=f"xt{s}", name=f"xt{s}")
          for s in range(CS)]

    # ---- phase 1: x^T strips (x is pre-transposed host-side) ----
    # halves spread across more DMA queues; issue alternates between
    # the two HWDGE engines (Sync, ACT) so descriptor generation for
    # the critical x^T stream is not serialized behind one sequencer
    for s in range(CS):
        for h in range(2):
            eng = nc.sync if (2 * s + h) % 2 == 0 else nc.scalar
            eng.dma_start(xt[s][:, h * 1024:(h + 1) * 1024],
                          x[s * 128:(s + 1) * 128,
                            h * 1024:(h + 1) * 1024])
    # b_attn per-partition bias columns for the q|k strips
    bias_qk = const_pool.tile([128, 8], F32, tag="biasqk")
    nc.sync.dma_start(bias_qk[:], bqk.rearrange("(s p) -> p s", p=128))
    # b_attn for the v columns, replicated down all 128 partitions so
    # the vau evacuation can add it as a plain tensor operand
    bv = const_pool.tile([128, NV], BF16, tag="bv")
    nc.sync.dma_start(bv[:], wqkv[C:C + 1, NQK:].broadcast_to([128, NV]))

    # ---- phases 2+3: qkv / attention / proj, filler-interleaved ----
    # PE executes in emission order, so the PE-dense qkv chains and the
    # ACT-paced attention must be woven together: attention runs
    # hp-outer (head-pair hp needs only strips hp, 4+hp), and a filler
    # queue drips one qkv chain / vau tile / proj chunk between
    # attention j-tiles. All chain psums share one 3-slot pool; the
    # psy pool (2 slots) doubles as proj psum. 8 banks total.
    wnn_pool = ctx.enter_context(tc.tile_pool(name="wnn", bufs=1))
    wn = [wnn_pool.tile([128, CS_AUG, 128], BF16, tag=f"wnn{nn}",
                        name=f"wnn{nn}")
          for nn in range(8)]
    wv_pool = ctx.enter_context(tc.tile_pool(name="wv", bufs=1))
    wv = wv_pool.tile([128, CS_AUG, NV], BF16, tag="wv")
    # DMA priority: the first chains need wn0, wn4 and wv; wp is only
    # needed ~200us in
    def wn_dma(nn):
        # per-K-strip: chain matmul s only waits for its own strip
        for s in range(CS_AUG):
            nc.sync.dma_start(
                wn[nn][:, s, :],
                wqkv[s * 128:(s + 1) * 128, nn * 128:(nn + 1) * 128])

    for nn in (0, 4):
        wn_dma(nn)
    for s in range(CS_AUG):
        nc.sync.dma_start(wv[:, s, :], wqkv[s * 128:(s + 1) * 128, NQK:])
    for nn in (1, 5, 2, 6, 3, 7):
        wn_dma(nn)
    wp_pool = ctx.enter_context(tc.tile_pool(name="wp", bufs=1))
    wp = wp_pool.tile([128, NV // 128, C], BF16, tag="wp")
    nc.sync.dma_start(wp[:], wproj.rearrange("(s p) n -> p s n", p=128))

    pt_sb_pool = ctx.enter_context(tc.tile_pool(name="ptile", bufs=6))
    n_sb_pool = ctx.enter_context(tc.tile_pool(name="ntile", bufs=3))
    osb_pool = ctx.enter_context(tc.tile_pool(name="osb", bufs=2))
    ps_s_pool = ctx.enter_context(
        tc.tile_pool(name="ps_s", bufs=3, space="PSUM"))
    ps_y_pool = ctx.enter_context(
        tc.tile_pool(name="ps_y", bufs=1, space="PSUM"))

    def qk_chain(nn, tb):
        # one [128, 512] block of qkT strip nn (borrows a pss slot)
        ps = ps_s_pool.tile([128, 2, 512], F32, tag="pss",
                            name=f"pqk{nn}_{tb}")
        for s in range(CS):
            nc.tensor.matmul(ps[:, 0, :], wn[nn][:, s, :],
                             xt[s][:, tb * 512:(tb + 1) * 512],
                             start=(s == 0), stop=(s == CS - 1))
        # evacuate with the b_attn bias folded in (per-partition)
        nc.vector.tensor_scalar_add(
            qkt[nn][:, tb * 512:(tb + 1) * 512], ps[:, 0, :],
            bias_qk[:, nn:nn + 1])

    def vau_tile(tt):
        ps = ps_s_pool.tile([128, 2, 512], F32, tag="pss",
                            name=f"pv{tt}")
        for s in range(CS):
            nc.tensor.matmul(ps[:, 0, :],
                             xt[s][:, tt * 128:(tt + 1) * 128],
                             wv[:, s, :],
                             start=(s == 0), stop=(s == CS - 1))
        # evacuate with b_attn(v) added elementwise
        nc.vector.scalar_tensor_tensor(
            vau[tt][:, :, DH:],
            ps[:, 0, :].rearrange("p (h d) -> p h d", d=DH),
            0.0,
            bv[:].rearrange("p (h d) -> p h d", d=DH),
            mybir.AluOpType.add,
            mybir.AluOpType.add)

    osb = [osb_pool.tile([128, C], F32, tag=f"osb{i}", name=f"osb{i}")
           for i in range(2)]

    def proj_chunk(tt, nb):
        ps = ps_s_pool.tile([128, 2, 512], F32, tag="pss",
                            name=f"po{tt}_{nb}")
        for s in range(NV // 128):
            nc.tensor.matmul(
                ps[:, 0, :],
                yt[s][:, tt * 128:(tt + 1) * 128],
                wp[:, s, nb * 512:(nb + 1) * 512],
                start=(s == 0), stop=(s == NV // 128 - 1))
        o_sb = osb[tt % 2]
        nc.vector.tensor_copy(o_sb[:, nb * 512:(nb + 1) * 512],
                              ps[:, 0, :])
        if nb == C // 512 - 1:
            nc.sync.dma_start(out[tt * 128:(tt + 1) * 128, :], o_sb[:])

    def attn_gen(ib, hp):
        # generator: yields after each j-tile so fillers can interleave
        isl = slice(ib * 512, (ib + 1) * 512)
        jmax = 4 * ib + 3
        qs = qkt[hp]              # q strip: heads (2hp, 2hp+1)
        ks = qkt[4 + hp]          # k strip
        ps_y = [ps_y_pool.tile([128, 512], F32, tag=f"psy{u}",
                               name=f"psy{u}_{hp}_{ib}")
                for u in range(2)]

        def s_exp(jj):
            off = max(0, 128 * (jj - 4 * ib))
            ps_s = ps_s_pool.tile([128, 2, 512], F32, tag="pss")
            for u in range(2):   # head-pair halves: base 0 / 64
                plo = 64 * u
                nc.tensor.matmul(
                    ps_s[:, u, off:],
                    ks[plo:plo + DH, jj * 128:(jj + 1) * 128],
                    qs[plo:plo + DH, ib * 512 + off:(ib + 1) * 512],
                    start=True, stop=True)
            p = pt_sb_pool.tile([128, 2, 512], BF16, tag="pt")
            nc.scalar.activation(p[:, :, off:], ps_s[:, :, off:],
                                 ACT_EXP, scale=SCALE)
            if jj >= 4 * ib:       # diagonal tile: zero i < j
                nc.vector.tensor_mul(
                    p[:, :, off:off + 128],
                    p[:, :, off:off + 128],
                    mask01[:, None, :].broadcast_to([128, 2, 128]))
            return p

        def pv(jj, p):
            off = max(0, 128 * (jj - 4 * ib))
            for u in range(2):
                nc.tensor.matmul(ps_y[u][:, off:],
                                 vau[jj][:, 2 * hp + u, :],
                                 p[:, u, off:],
                                 start=(jj == 0), stop=(jj == jmax))

        # software pipeline, skew 2: S(jj+2) issues before PV(jj)
        p0 = s_exp(0)
        yield
        p1 = s_exp(1)
        yield
        for jj in range(2, jmax + 1):
            p2 = s_exp(jj)
            pv(jj - 2, p0)
            p0, p1 = p1, p2
            yield
        yield "tail"
        pv(jmax - 1, p0)
        pv(jmax, p1)
        for u in range(2):
            plo = 64 * u
            rbb = n_sb_pool.tile([64, 512], F32, tag=f"rbb{u}")
            nc.vector.reciprocal_approx_fast(
                out=rbb[:], in_=ps_y[u][0:64, :])
            nc.vector.tensor_mul(yt[hp][plo:plo + DH, isl],
                                 ps_y[u][64:128, :], rbb[:])

    # lead-in: strips for hp=0 + the first vau tiles (ACT idle anyway)
    for tb in range(TB):
        qk_chain(0, tb)
    for tb in range(TB):
        qk_chain(4, tb)
    for tt in range(0, 4):
        vau_tile(tt)

    # filler queue, ordered by when attention first needs each item
    fillers = []
    for tt in range(4, TT):
        fillers.append(lambda tt=tt: vau_tile(tt))
    for hp_next in (1, 2, 3):
        for nn in (hp_next, 4 + hp_next):
            for tb in range(TB):
                fillers.append(lambda nn=nn, tb=tb: qk_chain(nn, tb))

    fi = 0
    tick = 0
    pending = None     # deferred (gen, ib, hp) tail: last PVs + norm

    def emit_tail(entry):
        g, t_ib, t_hp = entry
        for _ in g:
            pass
        if t_hp == HL // 2 - 1:
            # this i-block's projection unlocks once hp3 finishes it
            for tt in range(4 * t_ib, 4 * t_ib + 4):
                for nb in range(C // 512):
                    fillers.append(
                        lambda tt=tt, nb=nb: proj_chunk(tt, nb))

    for hp in range(HL // 2):
        rate = (2, 3, 2, 1)[hp]
        for ib in range(TB):
            g = attn_gen(ib, hp)
            next(g)    # head: S(0)+exp(0) feeds ACT across the boundary
            tick += 1
            if tick % rate == 0 and fi < len(fillers):
                fillers[fi]()
                fi += 1
            if pending is not None:
                emit_tail(pending)
                pending = None
            while True:
                v = next(g)
                if v == "tail":
                    pending = (g, ib, hp)
                    break
                tick += 1
                if tick % rate == 0 and fi < len(fillers):
                    fillers[fi]()
                    fi += 1
    if pending is not None:
        emit_tail(pending)
    while fi < len(fillers):   # drain: remaining proj chunks
        fillers[fi]()
        fi += 1

_BUILD_LOCK = threading.Lock()
_CACHED = {}


def build_nc(repeat=1):
    with _BUILD_LOCK:
        if repeat in _CACHED:
            return _CACHED[repeat]
        nc = bacc.Bacc("TRN2", debug=False)
        x = nc.dram_tensor("x", [C, T], BF16, kind="ExternalInput").ap()
        wqkv = nc.dram_tensor("wqkv", [CS_AUG * 128, 3 * NV], BF16,
                              kind="ExternalInput").ap()
        wproj = nc.dram_tensor("wproj", [NV, C], BF16,
                               kind="ExternalInput").ap()
        bqk = nc.dram_tensor("bqk", [NQK], F32, kind="ExternalInput").ap()
        out = nc.dram_tensor("out", [T, C], F32, kind="ExternalOutput").ap()
        with tile.TileContext(nc, pool_alloc_mode="queue") as tc:
            for _ in range(repeat):
                with ExitStack() as ctx:
                    build_attention_kernel(ctx, tc, x, wqkv, wproj, bqk, out)
        nc.compile()
        _CACHED[repeat] = nc
        return nc


def shard_inputs(x, w_attn, b_attn, w_proj, b_proj):
    """Build the per-core input maps (numpy, bf16)."""
    x = np.asarray(x, dtype=np.float32)
    w_attn = np.asarray(w_attn, dtype=np.float32)
    b_attn = np.asarray(b_attn, dtype=np.float32)
    w_proj = np.asarray(w_proj, dtype=np.float32)
    in_maps = []
    for c in range(N_CORES):
        b, hh = divmod(c, 2)
        cols = np.r_[hh * 512:(hh + 1) * 512,
                     C + hh * 512:C + (hh + 1) * 512,
                     2 * C + hh * 512:2 * C + (hh + 1) * 512]
        w_aug = np.zeros((CS_AUG * 128, 3 * NV), np.float32)
        w_aug[:C] = w_attn[:, cols]
        w_aug[C] = b_attn[cols]
        in_maps.append({
            "x": np.ascontiguousarray(x[b].T).astype(NP_BF16),
            "wqkv": w_aug.astype(NP_BF16),
            "wproj": np.ascontiguousarray(
                w_proj[hh * 512:(hh + 1) * 512]).astype(NP_BF16),
            "bqk": np.ascontiguousarray(b_attn[cols[:NQK]]),
        })
    return in_maps


def kernel(x, w_attn, b_attn, w_proj, b_proj, _profile=False, _tmpdir=None):
    nc = build_nc()
    in_maps = shard_inputs(x, w_attn, b_attn, w_proj, b_proj)
    res = run_bass_kernel_spmd(nc, in_maps, list(range(N_CORES)),
                               trace=_profile, tmpdir=_tmpdir)
    b_proj = np.asarray(b_proj, dtype=np.float32)
    out = np.empty((B, T, C), np.float32)
    for b in range(B):
        out[b] = res.results[2 * b]["out"] + res.results[2 * b + 1]["out"] \
            + b_proj[None, :]
    if _profile:
        return out, res
    return out
